# revision 9
# baseline (speedup 1.0000x reference)
"""Causal multi-head attention (B=2, S=2048, D=1024, H=16, Dh=64) on 8 trn2 cores.

Sharding: (batch, head-group) tensor parallel. Core c handles batch c//4 and
heads [4*(c%4), 4*(c%4)+4). Each core computes its 4 heads end-to-end
(QKV projections, causal softmax attention, W_O projection) and returns a
partial [S, D] output; the host sums the 4 partials per batch.

Per-core dataflow (all matmuls in float32r for full PE rate):
  - Q^T, K^T produced in [Dh, S] layout so scores come out transposed
    (S^T[k, q]) and the softmax'd P~ needs no transpose for the P@V matmul.
  - Softmax denominator via a ones-column appended to V (M=65 matmuls):
    row 64 of the attention PSUM is the denominator.
  - Causal mask = additive -1e30 upper-triangular tile applied to the
    diagonal PSUM scores blocks before exp.
  - W_O projection consumes the normalized attn^T directly as the stationary
    operand, accumulating both head-pairs in PSUM.

Software pipelining (PE engine queues are in-order, so program order matters):
  - attn matmul for kt is emitted one kt behind its scores/exp, so the PE
    never stalls waiting for the ACT exp of the block it just scored.
  - W_O projection of q-chunk qc is emitted inside the attention loop of
    qc+1, after the normalize of qc has had a full chunk's time to finish.
  - x is loaded in s-quarters so projections start ~7us in, not after the
    full 8MB load.

float32r rules (walrus birverifier + ISA): any engine may *write* a f32r
tile (that is the rounding point), matmuls may read f32r, but DVE
tensor-tensor ops may not *read* f32r.
"""

import numpy as np

try:
    import concourse  # noqa: F401
except ImportError:  # pragma: no cover - harness containers stage it here
    import sys

    sys.path.insert(0, "/opt/trn_rl_repo")

B, S, D, H, DH = 2, 2048, 1024, 16, 64
NCORES = 8
HPC = 4  # heads per core
NPAIR = 2  # head pairs per core
SC = 512  # q-chunk width (scores matmul N)
NQC = S // SC  # 4 q-chunks
NST = S // 128  # 16 s/k/q tiles of 128
NDC = D // 128  # 8 contraction chunks of 128
VO_W = 65  # V columns + ones column
VO_STRIDE = NST * VO_W  # per-head column stride in the V|ones tile

_cache = {}


def _build_program():
    from contextlib import ExitStack

    import concourse.mybir as mybir
    import concourse.tile as tile
    from concourse import bacc

    f32 = mybir.dt.float32
    f32r = mybir.dt.float32r
    AF = mybir.ActivationFunctionType

    nc = bacc.Bacc(
        "TRN2", debug=False, target_bir_lowering=False, num_devices=NCORES
    )

    xT = nc.dram_tensor("xT", [D, S], f32r, kind="ExternalInput").ap()
    wqk = nc.dram_tensor(
        "wqk", [128, 4 * NDC * 128], f32r, kind="ExternalInput"
    ).ap()
    wv = nc.dram_tensor("wv", [128, NDC * 256], f32r, kind="ExternalInput").ap()
    wo = nc.dram_tensor("wo", [128, NPAIR * D], f32r, kind="ExternalInput").ap()
    tri = nc.dram_tensor("tri", [128, 128], f32, kind="ExternalInput").ap()
    out = nc.dram_tensor("out", [S, D], f32, kind="ExternalOutput").ap()

    with tile.TileContext(nc) as tc, ExitStack() as ctx:
        persist = ctx.enter_context(tc.tile_pool(name="persist", bufs=1))
        pt_pool = ctx.enter_context(tc.tile_pool(name="pt", bufs=6))
        den_pool = ctx.enter_context(tc.tile_pool(name="den", bufs=2))
        out_pool = ctx.enter_context(tc.tile_pool(name="outsb", bufs=2))
        ps_pool = ctx.enter_context(tc.tile_pool(name="ps", bufs=4, space="PSUM"))
        pa_pool = ctx.enter_context(tc.tile_pool(name="pa", bufs=4, space="PSUM"))

        # ---- persistent SBUF tensors ----
        x_sb = {
            (dc, q): persist.tile(
                [128, SC], f32r, tag=f"x{dc}_{q}", name=f"x{dc}_{q}"
            )
            for dc in range(NDC)
            for q in range(NQC)
        }
        wqk_sb = persist.tile([128, 4 * NDC * 128], f32r, tag="wqk", name="wqk_sb")
        wv_sb = persist.tile([128, NDC * 256], f32r, tag="wv", name="wv_sb")
        wo_sb = persist.tile([128, NPAIR * D], f32r, tag="wo", name="wo_sb")
        tri_sb = persist.tile([128, 128], f32, tag="tri", name="tri_sb")
        ones_sb = persist.tile([128, 1], f32, tag="ones", name="ones_sb")
        qt_sb = [
            persist.tile([128, S], f32r, tag=f"qt{p}", name=f"qt{p}")
            for p in range(NPAIR)
        ]
        kt_sb = [
            persist.tile([128, S], f32r, tag=f"kt{p}", name=f"kt{p}")
            for p in range(NPAIR)
        ]
        vo_sb = persist.tile([128, HPC * VO_STRIDE], f32r, tag="vo", name="vo_sb")
        at_sb = {
            (p, qc): persist.tile(
                [128, SC], f32r, tag=f"at{p}_{qc}", name=f"at{p}_{qc}"
            )
            for p in range(NPAIR)
            for qc in range(NQC)
        }

        # ---- loads (x in s-quarters so compute starts early) ----
        nc.sync.dma_start(wqk_sb[:], wqk[:])
        nc.sync.dma_start(wv_sb[:], wv[:])
        nc.sync.dma_start(wo_sb[:], wo[:])
        nc.sync.dma_start(tri_sb[:], tri[:])
        for q in range(NQC):
            for dc in range(NDC):
                nc.sync.dma_start(
                    x_sb[(dc, q)][:],
                    xT[dc * 128 : (dc + 1) * 128, q * SC : (q + 1) * SC],
                )
        nc.vector.memset(ones_sb[:], 1.0)
        for h in range(HPC):
            head_cols = vo_sb[:, h * VO_STRIDE : (h + 1) * VO_STRIDE]
            ones_cols = head_cols.rearrange("p (s w) -> p s w", w=VO_W)[:, :, 64]
            nc.vector.tensor_copy(ones_cols, ones_sb[:].to_broadcast((128, NST)))

        # ---- QKV projections, by s-quarter ----
        for q in range(NQC):
            for p in range(NPAIR):
                for qk, dst in ((0, qt_sb[p]), (1, kt_sb[p])):
                    ps = ps_pool.tile(
                        [128, SC], f32, tag="ps", name=f"psqk{p}{qk}{q}"
                    )
                    for dc in range(NDC):
                        col = ((qk * NPAIR + p) * NDC + dc) * 128
                        nc.tensor.matmul(
                            ps[:],
                            lhsT=wqk_sb[:, col : col + 128],
                            rhs=x_sb[(dc, q)][:],
                            start=(dc == 0),
                            stop=(dc == NDC - 1),
                        )
                    nc.scalar.copy(dst[:, q * SC : (q + 1) * SC], ps[:])
            for st4 in range(4):
                st = q * 4 + st4
                ps = ps_pool.tile([128, 256], f32, tag="ps", name=f"psv{st}")
                for dc in range(NDC):
                    nc.tensor.matmul(
                        ps[:],
                        lhsT=x_sb[(dc, q)][:, st4 * 128 : (st4 + 1) * 128],
                        rhs=wv_sb[:, dc * 256 : (dc + 1) * 256],
                        start=(dc == 0),
                        stop=(dc == NDC - 1),
                    )
                for h in range(HPC):
                    base = h * VO_STRIDE + st * VO_W
                    nc.vector.tensor_copy(
                        vo_sb[:, base : base + 64], ps[:, h * 64 : (h + 1) * 64]
                    )

        # ---- attention (1-kt software pipeline) + deferred W_O ----
        def emit_wo(qc):
            for qt in range(4):
                po = [
                    ps_pool.tile([128, SC], f32, tag="ps", name=f"po{qc}{qt}{dc}")
                    for dc in range(2)
                ]
                for p in range(NPAIR):
                    for dc in range(2):
                        nc.tensor.matmul(
                            po[dc][:],
                            lhsT=at_sb[(p, qc)][:, qt * 128 : (qt + 1) * 128],
                            rhs=wo_sb[:, p * D + dc * SC : p * D + (dc + 1) * SC],
                            start=(p == 0),
                            stop=(p == NPAIR - 1),
                        )
                outt = out_pool.tile([128, D], f32, tag="outsb", name=f"o{qc}{qt}")
                for dc in range(2):
                    nc.scalar.copy(outt[:, dc * SC : (dc + 1) * SC], po[dc][:])
                row = (qc * 4 + qt) * 128
                nc.sync.dma_start(out[row : row + 128, :], outt[:])

        for qc in range(NQC):
            pa_qc = {}
            for p in range(NPAIR):
                pa = [
                    pa_pool.tile([VO_W, SC], f32, tag="pa", name=f"pa{qc}{p}{par}")
                    for par in range(2)
                ]
                pa_qc[p] = pa
                nkt = 4 * (qc + 1)
                pending = None  # (kt, par->ptile) awaiting attn matmul

                def flush(pend):
                    kt, ptiles = pend
                    j0 = max(0, kt * 128 - qc * SC)
                    for par in range(2):
                        hh = 2 * p + par
                        vbase = hh * VO_STRIDE + kt * VO_W
                        nc.tensor.matmul(
                            pa[par][:, j0:SC],
                            lhsT=vo_sb[:, vbase : vbase + VO_W],
                            rhs=ptiles[par][:, j0:SC],
                            start=(kt == 0),
                            stop=(kt == nkt - 1),
                        )

                for kt in range(nkt):
                    j0 = max(0, kt * 128 - qc * SC)
                    ptiles = []
                    for par in range(2):
                        ps_s = ps_pool.tile(
                            [128, SC], f32, tag="ps", name=f"pss{qc}{p}{kt}{par}"
                        )
                        nc.tensor.matmul(
                            ps_s[:, j0:SC],
                            lhsT=kt_sb[p][
                                par * 64 : (par + 1) * 64,
                                kt * 128 : (kt + 1) * 128,
                            ],
                            rhs=qt_sb[p][
                                par * 64 : (par + 1) * 64,
                                qc * SC + j0 : (qc + 1) * SC,
                            ],
                            start=True,
                            stop=True,
                        )
                        if kt * 128 >= qc * SC:  # diagonal block: causal mask
                            nc.vector.tensor_add(
                                ps_s[:, j0 : j0 + 128],
                                ps_s[:, j0 : j0 + 128],
                                tri_sb[:],
                            )
                        ptile = pt_pool.tile(
                            [128, SC], f32r, tag="pt", name=f"pt{qc}{p}{kt}{par}"
                        )
                        nc.scalar.activation(
                            ptile[:, j0:SC], ps_s[:, j0:SC], AF.Exp, scale=0.125
                        )
                        ptiles.append(ptile)
                    if pending is not None:
                        flush(pending)
                    pending = (kt, ptiles)
                flush(pending)
                # W_O of the previous q-chunk, between the two pairs' loops
                if p == 0 and qc > 0:
                    emit_wo(qc - 1)

            # normalize both pairs: one batched reciprocal for the 4 rows
            # (engine APs need partition bases that are multiples of 32, so
            # the rows live at partitions 0/32/64/96 of a [97, SC] tile)
            den = den_pool.tile([97, SC], f32, tag="den", name=f"den{qc}")
            nc.vector.memset(den[:], 1.0)
            for p in range(NPAIR):
                for par in range(2):
                    i = 32 * (2 * p + par)
                    nc.vector.tensor_copy(
                        den[i : i + 1, :], pa_qc[p][par][64:65, :]
                    )
            den_r = den_pool.tile([97, SC], f32, tag="denr", name=f"denr{qc}")
            nc.vector.reciprocal(den_r[:], den[:])
            for p in range(NPAIR):
                for par in range(2):
                    i = 32 * (2 * p + par)
                    # partition_broadcast HW ucode reads partition 0 of the
                    # source tile regardless of the AP base (sim honors the
                    # base) - bounce each row through a base-0 tile first
                    den_s = den_pool.tile(
                        [1, SC], f32, tag="dens", name=f"dens{qc}{p}{par}"
                    )
                    nc.vector.tensor_copy(den_s[:], den_r[i : i + 1, :])
                    denb = den_pool.tile(
                        [64, SC], f32, tag="denb", name=f"denb{qc}{p}{par}"
                    )
                    nc.gpsimd.partition_broadcast(denb[:], den_s[:])
                    nc.vector.tensor_mul(
                        at_sb[(p, qc)][par * 64 : (par + 1) * 64, :],
                        pa_qc[p][par][0:64, :],
                        denb[:],
                    )
        emit_wo(NQC - 1)

    nc.compile()
    return nc


def _get_program():
    if "nc" not in _cache:
        _cache["nc"] = _build_program()
    return _cache["nc"]


def _prep_core_inputs(c, residual, W_Q, W_K, W_V, W_O, tri):
    b = c // 4
    heads = [4 * (c % 4) + i for i in range(HPC)]

    def chunked(w):  # [1024, M] -> [128, NDC*M] chunk-major
        m = w.shape[1]
        return np.ascontiguousarray(
            w.reshape(NDC, 128, m).transpose(1, 0, 2).reshape(128, NDC * m)
        )

    wqk_blocks = []
    for Wt in (W_Q, W_K):
        for p in range(NPAIR):
            h0, h1 = heads[2 * p], heads[2 * p + 1]
            wpair = np.concatenate([Wt[h0].T, Wt[h1].T], axis=1)  # [1024, 128]
            wqk_blocks.append(chunked(wpair))
    wqk_arr = np.ascontiguousarray(np.concatenate(wqk_blocks, axis=1))

    wv_arr = chunked(np.concatenate([W_V[h].T for h in heads], axis=1))
    wo_arr = np.ascontiguousarray(
        np.concatenate(
            [
                np.concatenate([W_O[heads[2 * p]], W_O[heads[2 * p + 1]]], axis=0)
                for p in range(NPAIR)
            ],
            axis=1,
        )
    )
    return {
        "xT": np.ascontiguousarray(residual[b].T),
        "wqk": wqk_arr,
        "wv": wv_arr,
        "wo": wo_arr,
        "tri": tri,
    }


def make_in_maps(residual, W_Q, W_K, W_V, W_O):
    residual = np.asarray(residual, np.float32)
    W_Q, W_K, W_V, W_O = (np.asarray(w, np.float32) for w in (W_Q, W_K, W_V, W_O))
    # additive causal mask for S^T[k, q] diagonal blocks: keep j >= p
    tri = np.where(np.triu(np.ones((128, 128), bool)), 0.0, -1e30).astype(np.float32)
    return [
        _prep_core_inputs(c, residual, W_Q, W_K, W_V, W_O, tri)
        for c in range(NCORES)
    ]


def gather(results):
    out = np.zeros((B, S, D), np.float64)
    for c in range(NCORES):
        out[c // 4] += results[c]["out"].astype(np.float64)
    return out.astype(np.float32)


def kernel(residual, W_Q, W_K, W_V, W_O, **run_kwargs):
    from concourse.bass_utils import run_bass_kernel_spmd

    nc = _get_program()
    in_maps = make_in_maps(residual, W_Q, W_K, W_V, W_O)
    res = run_bass_kernel_spmd(nc, in_maps, core_ids=list(range(NCORES)), **run_kwargs)
    out = gather(res.results)
    if run_kwargs:
        _cache["last_results"] = res
    return out


# revision 10
# speedup vs baseline: 1.1491x; 1.1491x over previous
"""Causal multi-head attention (B=2, S=2048, D=1024, H=16, Dh=64) on 8 trn2 cores.

Sharding: (batch, head-group) tensor parallel. Core c handles batch c//4 and
heads [4*(c%4), 4*(c%4)+4). Each core computes its 4 heads end-to-end
(QKV projections, causal softmax attention, W_O projection) and returns a
partial [S, D] output; the host sums the 4 partials per batch.

Per-core dataflow (all matmuls in float32r for full PE rate):
  - Q^T, K^T produced in [Dh, S] layout so scores come out transposed
    (S^T[k, q]) and the softmax'd P~ needs no transpose for the P@V matmul.
  - Softmax denominator via a ones-column appended to V (M=65 matmuls):
    row 64 of the attention PSUM is the denominator.
  - Causal mask = additive -1e30 upper-triangular tile applied to the
    diagonal PSUM scores blocks before exp.
  - W_O projection consumes the normalized attn^T directly as the stationary
    operand, accumulating both head-pairs in PSUM.

Software pipelining (PE engine queues are in-order, so program order matters):
  - attn matmul for kt is emitted one kt behind its scores/exp, so the PE
    never stalls waiting for the ACT exp of the block it just scored.
  - W_O projection of q-chunk qc is emitted inside the attention loop of
    qc+1, after the normalize of qc has had a full chunk's time to finish.
  - x is loaded in s-quarters so projections start ~7us in, not after the
    full 8MB load.

float32r rules (walrus birverifier + ISA): any engine may *write* a f32r
tile (that is the rounding point), matmuls may read f32r, but DVE
tensor-tensor ops may not *read* f32r.
"""

import numpy as np

try:
    import concourse  # noqa: F401
except ImportError:  # pragma: no cover - harness containers stage it here
    import sys

    sys.path.insert(0, "/opt/trn_rl_repo")

B, S, D, H, DH = 2, 2048, 1024, 16, 64
NCORES = 8
HPC = 4  # heads per core
NPAIR = 2  # head pairs per core
SC = 512  # q-chunk width (scores matmul N)
NQC = S // SC  # 4 q-chunks
NST = S // 128  # 16 s/k/q tiles of 128
NDC = D // 128  # 8 contraction chunks of 128
VO_W = 65  # V columns + ones column
VO_STRIDE = NST * VO_W  # per-head column stride in the V|ones tile

_cache = {}


def _build_program():
    from contextlib import ExitStack

    import concourse.mybir as mybir
    import concourse.tile as tile
    from concourse import bacc

    f32 = mybir.dt.float32
    f32r = mybir.dt.float32r
    bf16 = mybir.dt.bfloat16
    AF = mybir.ActivationFunctionType

    nc = bacc.Bacc(
        "TRN2", debug=False, target_bir_lowering=False, num_devices=NCORES
    )

    xT = nc.dram_tensor("xT", [D, S], bf16, kind="ExternalInput").ap()
    wqk = nc.dram_tensor(
        "wqk", [128, 4 * NDC * 128], bf16, kind="ExternalInput"
    ).ap()
    wv = nc.dram_tensor("wv", [128, NDC * 256], bf16, kind="ExternalInput").ap()
    wo = nc.dram_tensor("wo", [128, NPAIR * D], f32r, kind="ExternalInput").ap()
    tri = nc.dram_tensor("tri", [128, 128], f32, kind="ExternalInput").ap()
    out = nc.dram_tensor("out", [S, D], f32, kind="ExternalOutput").ap()

    with tile.TileContext(nc) as tc, ExitStack() as ctx:
        persist = ctx.enter_context(tc.tile_pool(name="persist", bufs=1))
        pt_pool = ctx.enter_context(tc.tile_pool(name="pt", bufs=6))
        den_pool = ctx.enter_context(tc.tile_pool(name="den", bufs=2))
        out_pool = ctx.enter_context(tc.tile_pool(name="outsb", bufs=2))
        ps_pool = ctx.enter_context(tc.tile_pool(name="ps", bufs=4, space="PSUM"))
        pa_pool = ctx.enter_context(tc.tile_pool(name="pa", bufs=4, space="PSUM"))

        # ---- persistent SBUF tensors ----
        x_sb = {
            (dc, q): persist.tile(
                [128, SC], bf16, tag=f"x{dc}_{q}", name=f"x{dc}_{q}"
            )
            for dc in range(NDC)
            for q in range(NQC)
        }
        wqk_sb = persist.tile([128, 4 * NDC * 128], bf16, tag="wqk", name="wqk_sb")
        wv_sb = persist.tile([128, NDC * 256], bf16, tag="wv", name="wv_sb")
        wo_sb = persist.tile([128, NPAIR * D], f32r, tag="wo", name="wo_sb")
        tri_sb = persist.tile([128, 128], f32, tag="tri", name="tri_sb")
        ones_sb = persist.tile([128, 1], f32, tag="ones", name="ones_sb")
        qt_sb = [
            persist.tile([128, S], bf16, tag=f"qt{p}", name=f"qt{p}")
            for p in range(NPAIR)
        ]
        kt_sb = [
            persist.tile([128, S], bf16, tag=f"kt{p}", name=f"kt{p}")
            for p in range(NPAIR)
        ]
        vo_sb = persist.tile([128, HPC * VO_STRIDE], bf16, tag="vo", name="vo_sb")
        at_sb = {
            (p, qc): persist.tile(
                [128, SC], f32r, tag=f"at{p}_{qc}", name=f"at{p}_{qc}"
            )
            for p in range(NPAIR)
            for qc in range(NQC)
        }

        # ---- loads (x in s-quarters so compute starts early) ----
        nc.sync.dma_start(wqk_sb[:], wqk[:])
        nc.sync.dma_start(wv_sb[:], wv[:])
        nc.sync.dma_start(wo_sb[:], wo[:])
        nc.sync.dma_start(tri_sb[:], tri[:])
        for q in range(NQC):
            for dc in range(NDC):
                nc.sync.dma_start(
                    x_sb[(dc, q)][:],
                    xT[dc * 128 : (dc + 1) * 128, q * SC : (q + 1) * SC],
                )
        nc.vector.memset(ones_sb[:], 1.0)
        for h in range(HPC):
            head_cols = vo_sb[:, h * VO_STRIDE : (h + 1) * VO_STRIDE]
            ones_cols = head_cols.rearrange("p (s w) -> p s w", w=VO_W)[:, :, 64]
            nc.vector.tensor_copy(ones_cols, ones_sb[:].to_broadcast((128, NST)))

        # ---- QKV projections, by s-quarter ----
        for q in range(NQC):
            for p in range(NPAIR):
                for qk, dst in ((0, qt_sb[p]), (1, kt_sb[p])):
                    ps = ps_pool.tile(
                        [128, SC], f32, tag="ps", name=f"psqk{p}{qk}{q}"
                    )
                    for dc in range(NDC):
                        col = ((qk * NPAIR + p) * NDC + dc) * 128
                        nc.tensor.matmul(
                            ps[:],
                            lhsT=wqk_sb[:, col : col + 128],
                            rhs=x_sb[(dc, q)][:],
                            start=(dc == 0),
                            stop=(dc == NDC - 1),
                        )
                    nc.scalar.copy(dst[:, q * SC : (q + 1) * SC], ps[:])
            for st4 in range(4):
                st = q * 4 + st4
                ps = ps_pool.tile([128, 256], f32, tag="ps", name=f"psv{st}")
                for dc in range(NDC):
                    nc.tensor.matmul(
                        ps[:],
                        lhsT=x_sb[(dc, q)][:, st4 * 128 : (st4 + 1) * 128],
                        rhs=wv_sb[:, dc * 256 : (dc + 1) * 256],
                        start=(dc == 0),
                        stop=(dc == NDC - 1),
                    )
                for h in range(HPC):
                    base = h * VO_STRIDE + st * VO_W
                    nc.vector.tensor_copy(
                        vo_sb[:, base : base + 64], ps[:, h * 64 : (h + 1) * 64]
                    )

        # ---- attention (1-kt software pipeline) + deferred W_O ----
        def emit_wo(qc):
            for qt in range(4):
                po = [
                    ps_pool.tile([128, SC], f32, tag="ps", name=f"po{qc}{qt}{dc}")
                    for dc in range(2)
                ]
                for p in range(NPAIR):
                    for dc in range(2):
                        nc.tensor.matmul(
                            po[dc][:],
                            lhsT=at_sb[(p, qc)][:, qt * 128 : (qt + 1) * 128],
                            rhs=wo_sb[:, p * D + dc * SC : p * D + (dc + 1) * SC],
                            start=(p == 0),
                            stop=(p == NPAIR - 1),
                        )
                outt = out_pool.tile([128, D], f32, tag="outsb", name=f"o{qc}{qt}")
                for dc in range(2):
                    nc.scalar.copy(outt[:, dc * SC : (dc + 1) * SC], po[dc][:])
                row = (qc * 4 + qt) * 128
                nc.sync.dma_start(out[row : row + 128, :], outt[:])

        for qc in range(NQC):
            pa_qc = {}
            for p in range(NPAIR):
                pa = [
                    pa_pool.tile([VO_W, SC], f32, tag="pa", name=f"pa{qc}{p}{par}")
                    for par in range(2)
                ]
                pa_qc[p] = pa
                nkt = 4 * (qc + 1)
                pending = None  # (kt, par->ptile) awaiting attn matmul

                def flush(pend):
                    kt, ptiles = pend
                    j0 = max(0, kt * 128 - qc * SC)
                    for par in range(2):
                        hh = 2 * p + par
                        vbase = hh * VO_STRIDE + kt * VO_W
                        nc.tensor.matmul(
                            pa[par][:, j0:SC],
                            lhsT=vo_sb[:, vbase : vbase + VO_W],
                            rhs=ptiles[par][:, j0:SC],
                            start=(kt == 0),
                            stop=(kt == nkt - 1),
                        )

                for kt in range(nkt):
                    j0 = max(0, kt * 128 - qc * SC)
                    ptiles = []
                    for par in range(2):
                        ps_s = ps_pool.tile(
                            [128, SC], f32, tag="ps", name=f"pss{qc}{p}{kt}{par}"
                        )
                        nc.tensor.matmul(
                            ps_s[:, j0:SC],
                            lhsT=kt_sb[p][
                                par * 64 : (par + 1) * 64,
                                kt * 128 : (kt + 1) * 128,
                            ],
                            rhs=qt_sb[p][
                                par * 64 : (par + 1) * 64,
                                qc * SC + j0 : (qc + 1) * SC,
                            ],
                            start=True,
                            stop=True,
                        )
                        if kt * 128 >= qc * SC:  # diagonal block: causal mask
                            nc.vector.tensor_add(
                                ps_s[:, j0 : j0 + 128],
                                ps_s[:, j0 : j0 + 128],
                                tri_sb[:],
                            )
                        ptile = pt_pool.tile(
                            [128, SC], bf16, tag="pt", name=f"pt{qc}{p}{kt}{par}"
                        )
                        nc.scalar.activation(
                            ptile[:, j0:SC], ps_s[:, j0:SC], AF.Exp, scale=0.125
                        )
                        ptiles.append(ptile)
                    if pending is not None:
                        flush(pending)
                    pending = (kt, ptiles)
                flush(pending)
                # W_O of the previous q-chunk, between the two pairs' loops
                if p == 0 and qc > 0:
                    emit_wo(qc - 1)

            # normalize both pairs: one batched reciprocal for the 4 rows
            # (engine APs need partition bases that are multiples of 32, so
            # the rows live at partitions 0/32/64/96 of a [97, SC] tile)
            den = den_pool.tile([97, SC], f32, tag="den", name=f"den{qc}")
            nc.vector.memset(den[:], 1.0)
            for p in range(NPAIR):
                for par in range(2):
                    i = 32 * (2 * p + par)
                    nc.vector.tensor_copy(
                        den[i : i + 1, :], pa_qc[p][par][64:65, :]
                    )
            den_r = den_pool.tile([97, SC], f32, tag="denr", name=f"denr{qc}")
            nc.vector.reciprocal(den_r[:], den[:])
            for p in range(NPAIR):
                for par in range(2):
                    i = 32 * (2 * p + par)
                    # partition_broadcast HW ucode reads partition 0 of the
                    # source tile regardless of the AP base (sim honors the
                    # base) - bounce each row through a base-0 tile first
                    den_s = den_pool.tile(
                        [1, SC], f32, tag="dens", name=f"dens{qc}{p}{par}"
                    )
                    nc.vector.tensor_copy(den_s[:], den_r[i : i + 1, :])
                    denb = den_pool.tile(
                        [64, SC], f32, tag="denb", name=f"denb{qc}{p}{par}"
                    )
                    nc.gpsimd.partition_broadcast(denb[:], den_s[:])
                    nc.vector.tensor_mul(
                        at_sb[(p, qc)][par * 64 : (par + 1) * 64, :],
                        pa_qc[p][par][0:64, :],
                        denb[:],
                    )
        emit_wo(NQC - 1)

    nc.compile()
    return nc


def _get_program():
    if "nc" not in _cache:
        _cache["nc"] = _build_program()
    return _cache["nc"]


def _prep_core_inputs(c, residual, W_Q, W_K, W_V, W_O, tri):
    b = c // 4
    heads = [4 * (c % 4) + i for i in range(HPC)]

    def chunked(w):  # [1024, M] -> [128, NDC*M] chunk-major
        m = w.shape[1]
        return np.ascontiguousarray(
            w.reshape(NDC, 128, m).transpose(1, 0, 2).reshape(128, NDC * m)
        )

    wqk_blocks = []
    for Wt in (W_Q, W_K):
        for p in range(NPAIR):
            h0, h1 = heads[2 * p], heads[2 * p + 1]
            wpair = np.concatenate([Wt[h0].T, Wt[h1].T], axis=1)  # [1024, 128]
            wqk_blocks.append(chunked(wpair))
    wqk_arr = np.ascontiguousarray(np.concatenate(wqk_blocks, axis=1))

    wv_arr = chunked(np.concatenate([W_V[h].T for h in heads], axis=1))
    wo_arr = np.ascontiguousarray(
        np.concatenate(
            [
                np.concatenate([W_O[heads[2 * p]], W_O[heads[2 * p + 1]]], axis=0)
                for p in range(NPAIR)
            ],
            axis=1,
        )
    )
    import ml_dtypes

    return {
        "xT": np.ascontiguousarray(residual[b].T).astype(ml_dtypes.bfloat16),
        "wqk": wqk_arr.astype(ml_dtypes.bfloat16),
        "wv": wv_arr.astype(ml_dtypes.bfloat16),
        "wo": wo_arr,
        "tri": tri,
    }


def make_in_maps(residual, W_Q, W_K, W_V, W_O):
    residual = np.asarray(residual, np.float32)
    W_Q, W_K, W_V, W_O = (np.asarray(w, np.float32) for w in (W_Q, W_K, W_V, W_O))
    # additive causal mask for S^T[k, q] diagonal blocks: keep j >= p
    tri = np.where(np.triu(np.ones((128, 128), bool)), 0.0, -1e30).astype(np.float32)
    return [
        _prep_core_inputs(c, residual, W_Q, W_K, W_V, W_O, tri)
        for c in range(NCORES)
    ]


def gather(results):
    out = np.zeros((B, S, D), np.float64)
    for c in range(NCORES):
        out[c // 4] += results[c]["out"].astype(np.float64)
    return out.astype(np.float32)


def kernel(residual, W_Q, W_K, W_V, W_O, **run_kwargs):
    from concourse.bass_utils import run_bass_kernel_spmd

    nc = _get_program()
    in_maps = make_in_maps(residual, W_Q, W_K, W_V, W_O)
    res = run_bass_kernel_spmd(nc, in_maps, core_ids=list(range(NCORES)), **run_kwargs)
    out = gather(res.results)
    if run_kwargs:
        _cache["last_results"] = res
    return out
